# revision 8
# baseline (speedup 1.0000x reference)
"""Trainium2 Bass kernel for nn_ConvolutionVoxel (e3nn voxel convolution).

The module computes  sc(x) + 0.1 * conv3d(x, K(tp_weight))  with a 5x5x5
e3nn tensor-product kernel on a (4, 32, 64, 64, 64) voxel grid.

Key observations exploited (all verified numerically):
  * The smooth-finite RBF basis has a hard cutoff at norm >= 1 and the
    kernel lattice spans [-1,1]^3, so the effective kernel is the central
    3x3x3 (26 nonzero taps) and the center tap is zero.
  * The self-connection is a 1x1x1 channel mix folded into the center tap.
  * Output = (large) center/self-connection part + (small, 0.1-scaled)
    26-tap part.  The small part tolerates fp8 weights/activations
    (max rel err contribution ~1e-3); the large part is done in bf16/fp16.

Mapping to the PE array (per core): contraction = 32 in-ch x (2z x 2h)
voxel block = 128, out = 32 out-ch x (2z x 2h) block = 128.  The 3 W-taps
are free-dim shifts of the rhs; z/h taps decompose into 2x2 neighbor
in-blocks.  The 12 resulting matmuls pair over bz into 6 fp8 DoubleRow
matmuls (contraction 256, 2x throughput), plus 1 bf16 matmul on an
*aligned* fp16 copy of x for the center tap + self-connection.
7 x 512 PE rows per tile instead of the all-bf16 12 x 512.

Sharding: 8 cores = 4 batches x 2 D-halves; 1 halo plane via host-side
zero padding (full inputs are given, so no device collectives needed).
"""
import numpy as np
import ml_dtypes

import concourse.bass as bass
import concourse.mybir as mybir
from concourse.bass_utils import run_bass_kernel_spmd

# ---------------------------------------------------------------------------
# host-side kernel construction (tiny: 5x256 weights -> the conv matrices)
# ---------------------------------------------------------------------------
MUL = 8
SIZE = 5
NUM_RBF = SIZE
PW_0E = (1.0 / 16.0) ** 0.5
PW_1O = (3.0 / 16.0) ** 0.5
INV_SQRT3 = 1.0 / (3.0 ** 0.5)
INV_SQRT_MUL = 1.0 / (MUL ** 0.5)


def _soft_unit_step(t):
    safe = np.where(t > 0, t, 1.0)
    return np.where(t > 0, np.exp(-1.0 / safe), 0.0)


def _build_K5(tp_weight):
    """(O,I,D,H,W) = (32,32,5,5,5) conv kernel, fp64."""
    tp_weight = np.asarray(tp_weight, np.float64)
    r = np.linspace(-1.0, 1.0, SIZE)
    lattice = np.stack(np.meshgrid(r, r, r, indexing='ij'), axis=-1)
    norm = np.linalg.norm(lattice, axis=-1)
    values = np.linspace(0.0, 1.0, NUM_RBF + 2)[1:-1]
    step = 1.0 / (NUM_RBF + 1)
    diff = (norm[..., None] - values) / step
    emb = 1.14136 * np.e ** 2 * _soft_unit_step(diff + 1.0) * _soft_unit_step(1.0 - diff)
    unit = lattice / np.maximum(norm, 1e-12)[..., None]
    sh1 = (3.0 ** 0.5) * unit[..., [1, 2, 0]]
    w = (emb @ tp_weight) / SIZE ** 1.5
    W1, W2, W3, W4 = [w[..., i * 64:(i + 1) * 64].reshape(SIZE, SIZE, SIZE, MUL, MUL)
                      for i in range(4)]
    B00 = PW_0E * W1
    B01 = (PW_1O * INV_SQRT3) * np.einsum('...uw,...k->...uwk', W2, sh1)
    B01 = B01.reshape(SIZE, SIZE, SIZE, MUL, 3 * MUL)
    B10 = (PW_0E * INV_SQRT3) * np.einsum('...uw,...i->...uiw', W4, sh1)
    B10 = B10.reshape(SIZE, SIZE, SIZE, 3 * MUL, MUL)
    B11 = (PW_1O * INV_SQRT3) * np.einsum('...uw,ik->...uiwk', W3, np.eye(3))
    B11 = B11.reshape(SIZE, SIZE, SIZE, 3 * MUL, 3 * MUL)
    K = np.concatenate([np.concatenate([B00, B01], axis=-1),
                        np.concatenate([B10, B11], axis=-1)], axis=-2)
    return K.transpose(4, 3, 0, 1, 2)


def _build_parts(tp_weight, w_sc0, w_sc1):
    """Returns (W12, Wc): W12 = 12 block-Toeplitz (128,128) matrices for the
    26 outer taps (center zeroed), Wc = (O,I) center tap + self-connection.

    W12 layout: rows p = ci*4 + zi*2 + hi (contraction), cols m =
    co*4 + zo*2 + ho.  t = dxi*4 + bz*2 + bh; dz = zi - zo + 2bz - 1,
    dh analogous, dw = dxi - 1.
    """
    K5 = _build_K5(tp_weight)
    K3 = 0.1 * K5[:, :, 1:4, 1:4, 1:4]
    Wsc = np.zeros((32, 32))
    Wsc[:8, :8] = INV_SQRT_MUL * np.asarray(w_sc0, np.float64)
    for i in range(3):
        Wsc[8 + i::3, 8 + i::3][:8, :8] = INV_SQRT_MUL * np.asarray(w_sc1, np.float64)
    Wc = K3[:, :, 1, 1, 1] + Wsc.T
    K3o = K3.copy()
    K3o[:, :, 1, 1, 1] = 0.0
    W = np.zeros((12, 128, 128))
    for dxi in range(3):
        for bz in range(2):
            for bh in range(2):
                t = dxi * 4 + bz * 2 + bh
                for zi in range(2):
                    for hi in range(2):
                        for zo in range(2):
                            for ho in range(2):
                                dz = zi - zo + 2 * bz - 1
                                dh = hi - ho + 2 * bh - 1
                                if abs(dz) <= 1 and abs(dh) <= 1:
                                    W[t, zi * 2 + hi::4, zo * 2 + ho::4] = \
                                        K3o[:, :, dz + 1, dh + 1, dxi].T
    return W, Wc


def _pack_w8(W12s):
    """(12,128,128) scaled fp64 -> (128, 6, 2, 128) e4m3.
    pair pr = dxi*2 + bh, k-tile dim = bz."""
    a = W12s.reshape(3, 2, 2, 128, 128).transpose(0, 2, 1, 3, 4)
    a = a.reshape(6, 2, 128, 128).transpose(2, 0, 1, 3)
    return np.ascontiguousarray(a).astype(ml_dtypes.float8_e4m3)


def _pack_wc(Wc, scale):
    """(O,I) center+sc -> (128,128) bf16 block-diagonal over the 4 voxel
    positions: wc[ci*4+q, co*4+q] = scale * Wc[co, ci]."""
    scaled = scale * Wc.T
    wcb = np.zeros((128, 128))
    for q in range(4):
        wcb[q::4, q::4] = scaled
    return np.ascontiguousarray(wcb).astype(ml_dtypes.bfloat16)


def _shard_input8(x8, core):
    """(128, 17, 33, 66) e4m3, partition-blocked and zero-padded.
    x8: full input already scaled+cast to e4m3.  core = 2*batch + d_half.
    p = ci*4 + zi*2 + hi; free (zb, hb, w) with z_in_local = 2*zb - 1 + zi,
    h = 2*hb - 1 + hi, w stored at w+1."""
    n, half = core // 2, core % 2
    d0 = 32 * half
    xp = np.zeros((32, 34, 66, 66), ml_dtypes.float8_e4m3)
    lo, hi = d0 - 1, d0 + 33
    glo, ghi = max(lo, 0), min(hi, 64)
    xp[:, glo - lo:ghi - lo, 1:65, 1:65] = x8[n, :, glo:ghi]
    return np.ascontiguousarray(
        xp.reshape(32, 17, 2, 33, 2, 66).transpose(0, 2, 4, 1, 3, 5)
        .reshape(128, 17, 33, 66))


def _shard_aligned(x16, core):
    """(128, 16, 2048) fp16, aligned blocking (no halo): p = ci*4+zi*2+hi,
    free (zb, hb*64+w) with z = 2*zb+zi, h = 2*hb+hi."""
    n, half = core // 2, core % 2
    d0 = 32 * half
    blk = x16[n, :, d0:d0 + 32]
    return np.ascontiguousarray(
        blk.reshape(32, 16, 2, 32, 2, 64).transpose(0, 2, 4, 1, 3, 5)
        .reshape(128, 16, 2048))


def _unshard_output(core_outs, descale):
    out = np.empty((4, 32, 64, 64, 64), np.float32)
    for c, oc in enumerate(core_outs):
        blk = np.asarray(oc).astype(np.float32) * descale
        blk = (blk.reshape(32, 2, 2, 16, 32, 64)
               .transpose(0, 3, 1, 4, 2, 5).reshape(32, 32, 64, 64))
        out[c // 2, :, 32 * (c % 2):32 * (c % 2) + 32] = blk
    return out


# ---------------------------------------------------------------------------
# device kernel
# ---------------------------------------------------------------------------
def _build_nc():
    """Raw-bass kernel (explicit semaphores).

    Engine streams: SP runs all DMAs (HWDGE), PE runs 448 matmuls
    (64 tiles x [1 bf16 center + 6 fp8 DoubleRow]), ACT evicts PSUM->SBUF
    as bf16.  Pipeline: 8 PSUM banks (2 zb groups) in flight; 8 output
    staging slots.  Per zb the 4 center matmuls are grouped before the 24
    DoubleRow matmuls to minimize PE dtype-mode switches."""
    nc = bass.Bass()
    x8 = nc.declare_dram_parameter("x8_sh", [128, 17, 33, 66], mybir.dt.float8e4,
                                   isOutput=False)
    xal = nc.declare_dram_parameter("xal_sh", [128, 16, 2048], mybir.dt.float16,
                                    isOutput=False)
    w8 = nc.declare_dram_parameter("w8", [128, 6, 2, 128], mybir.dt.float8e4,
                                   isOutput=False)
    wc = nc.declare_dram_parameter("wc", [128, 128], mybir.dt.bfloat16,
                                   isOutput=False)
    out = nc.declare_dram_parameter("out_sh", [128, 16, 32, 64], mybir.dt.bfloat16,
                                    isOutput=True)
    # Per-DMA semaphores: a DMA's completion increments its own sem by 16
    # (one per shard across the 16 HWDGE queues).  A shared counting sem
    # is racy: shard counts from a LATER dma can satisfy a threshold while
    # an earlier dma still has shards in flight on a lagging queue.
    wc_sem = nc.alloc_semaphore("wc_sem")
    w8_sem = nc.alloc_semaphore("w8_sem")
    xs_sem = [nc.alloc_semaphore(f"xs_sem{j}") for j in range(17)]
    al_sem = [nc.alloc_semaphore(f"al_sem{j}") for j in range(16)]
    od_sem = [nc.alloc_semaphore(f"od_sem{j}") for j in range(16)]
    warm_sem = nc.alloc_semaphore("warm_sem")
    N_WARM = 36  # PE pstate warmup matmuls covering the initial DMA wait
    with (
        nc.sbuf_tensor([128, 6, 2, 128], mybir.dt.float8e4) as wt8,
        nc.sbuf_tensor([128, 128], mybir.dt.bfloat16) as wtc,
        nc.sbuf_tensor([128, 17, 33, 66], mybir.dt.float8e4) as xsb,
        nc.sbuf_tensor([128, 16, 2048], mybir.dt.float16) as xab,
        nc.sbuf_tensor([128, 8, 2048], mybir.dt.bfloat16) as ot,
        nc.psum_tensor([128, 8, 512], mybir.dt.float32) as ps,
        nc.semaphore("pe_sem") as pe_sem,
        nc.semaphore("act_sem") as act_sem,
        nc.Block() as block,
    ):
        @block.vector
        def _(vector):
            # init the PE warmup scratch (ot slot 0); DVE is otherwise idle
            nc.vector.memset(ot[:, 0, :512], 0.0).then_inc(warm_sem, 1)

        @block.sync
        def _(sync):
            # order: first-center deps (wc, xal0), then first-DR deps
            sync.dma_start(out=wtc[:], in_=wc[:]).then_inc(wc_sem, 16)
            sync.dma_start(out=xab[:, 0], in_=xal[:, 0]).then_inc(al_sem[0], 16)
            sync.dma_start(out=wt8[:], in_=w8[:]).then_inc(w8_sem, 16)
            sync.dma_start(out=xsb[:, 0], in_=x8[:, 0]).then_inc(xs_sem[0], 16)
            sync.dma_start(out=xsb[:, 1], in_=x8[:, 1]).then_inc(xs_sem[1], 16)
            for j in range(1, 16):
                sync.dma_start(out=xab[:, j],
                               in_=xal[:, j]).then_inc(al_sem[j], 16)
                sync.dma_start(out=xsb[:, j + 1],
                               in_=x8[:, j + 1]).then_inc(xs_sem[j + 1], 16)
            for zb in range(16):
                sync.wait_ge(act_sem, 4 * (zb + 1))
                sync.dma_start(out=out[:, zb],
                               in_=ot[:, zb % 8]).then_inc(od_sem[zb], 16)
            for zb in range(16):
                sync.wait_ge(od_sem[zb], 16)

        @block.tensor
        def _(tensor):
            # pstate warmup on the ACT-memset scratch (ot slot 0); results
            # land in psum bank 0 and are overwritten by tile 0's start=True.
            tensor.wait_ge(warm_sem, 1)
            for _ in range(N_WARM):
                nc.tensor.matmul(ps[:, 0], lhsT=ot[:, 0, :128],
                                 rhs=ot[:, 0, :512], start=True, stop=True,
                                 skip_group_check=True)
            for zb in range(16):
                if zb == 0:
                    tensor.wait_ge(wc_sem, 16)
                tensor.wait_ge(al_sem[zb], 16)
                for ht in range(4):
                    i = zb * 4 + ht
                    if i >= 8:
                        # PSUM bank reuse: eviction of tile i-8 done
                        tensor.wait_ge(act_sem, i - 7)
                    nc.tensor.matmul(ps[:, i % 8], lhsT=wtc[:],
                                     rhs=xab[:, zb, ht * 512:(ht + 1) * 512],
                                     start=True, stop=False)
                if zb == 0:
                    tensor.wait_ge(w8_sem, 16)
                    tensor.wait_ge(xs_sem[0], 16)
                tensor.wait_ge(xs_sem[zb + 1], 16)
                for ht in range(4):
                    i = zb * 4 + ht
                    hb0 = ht * 8
                    for pr in range(6):
                        dxi, bh = pr // 2, pr % 2
                        dx = dxi - 1
                        rhs = xsb[:, zb:zb + 2, hb0 + bh:hb0 + bh + 8,
                                  1 + dx:65 + dx]
                        mm = nc.tensor.matmul(
                            ps[:, i % 8], lhsT=wt8[:, pr], rhs=rhs,
                            start=False, stop=(pr == 5),
                            perf_mode=mybir.MatmulPerfMode.DoubleRow)
                        if pr == 5:
                            mm.then_inc(pe_sem, 1)

        @block.scalar
        def _(scalar):
            for zb in range(16):
                for ht in range(4):
                    i = zb * 4 + ht
                    scalar.wait_ge(pe_sem, i + 1)
                    if ht == 0 and zb >= 8:
                        # output staging slot reuse: DMA of zb-8 done
                        scalar.wait_ge(od_sem[zb - 8], 16)
                    nc.scalar.copy(
                        out=ot[:, zb % 8, ht * 512:(ht + 1) * 512],
                        in_=ps[:, i % 8]).then_inc(act_sem, 1)

    return nc


_NC_CACHE = None


def _get_nc():
    global _NC_CACHE
    if _NC_CACHE is None:
        _NC_CACHE = _build_nc()
    return _NC_CACHE


def _pow2_scale(absmax, target=192.0):
    return float(2.0 ** np.floor(np.log2(target / max(absmax, 1e-30))))


def _prepare_inputs(x, tp_weight, w_sc0, w_sc1):
    """Returns (in_maps, descale)."""
    x = np.asarray(x, np.float32)
    s_x = _pow2_scale(float(np.abs(x).max()))
    W12, Wc = _build_parts(tp_weight, w_sc0, w_sc1)
    s_w = _pow2_scale(float(np.abs(W12).max()))
    w8 = _pack_w8(W12 * s_w)
    wcq = _pack_wc(Wc, s_w * s_x)
    x8full = (x * s_x).astype(ml_dtypes.float8_e4m3)
    x16full = x.astype(np.float16)
    in_maps = [{"x8_sh": _shard_input8(x8full, c),
                "xal_sh": _shard_aligned(x16full, c),
                "w8": w8, "wc": wcq} for c in range(8)]
    return in_maps, 1.0 / (s_w * s_x)


def run_sharded(x, tp_weight, w_sc0, w_sc1, **run_kwargs):
    """Shard, run on 8 cores, unshard.  Returns (output, BassKernelResults)."""
    in_maps, descale = _prepare_inputs(x, tp_weight, w_sc0, w_sc1)
    nc = _get_nc()
    res = run_bass_kernel_spmd(nc, in_maps, list(range(8)), **run_kwargs)
    outs = [res.results[c]["out_sh"] for c in range(8)]
    return _unshard_output(outs, descale), res


def kernel(x, tp_weight, w_sc0, w_sc1):
    out, _ = run_sharded(x, tp_weight, w_sc0, w_sc1)
    return out


# revision 12
# speedup vs baseline: 1.0222x; 1.0222x over previous
"""Trainium2 Bass kernel for nn_ConvolutionVoxel (e3nn voxel convolution).

The module computes  sc(x) + 0.1 * conv3d(x, K(tp_weight))  with a 5x5x5
e3nn tensor-product kernel on a (4, 32, 64, 64, 64) voxel grid.

Key observations exploited (all verified numerically):
  * The smooth-finite RBF basis has a hard cutoff at norm >= 1 and the
    kernel lattice spans [-1,1]^3, so the effective kernel is the central
    3x3x3 (26 nonzero taps) and the center tap is zero.
  * The self-connection is a 1x1x1 channel mix folded into the center tap.
  * Output = (large) center/self-connection part + (small, 0.1-scaled)
    26-tap part.  The small part tolerates fp8 weights/activations
    (max rel err contribution ~1e-3); the large part is done in bf16/fp16.

Mapping to the PE array (per core): contraction = 32 in-ch x (2z x 2h)
voxel block = 128, out = 32 out-ch x (2z x 2h) block = 128.  The 3 W-taps
are free-dim shifts of the rhs; z/h taps decompose into 2x2 neighbor
in-blocks.  The 12 resulting matmuls pair over bz into 6 fp8 DoubleRow
matmuls (contraction 256, 2x throughput), plus 1 bf16 matmul on an
*aligned* fp16 copy of x for the center tap + self-connection.
7 x 512 PE rows per tile instead of the all-bf16 12 x 512.

Sharding: 8 cores = 4 batches x 2 D-halves; 1 halo plane via host-side
zero padding (full inputs are given, so no device collectives needed).
"""
import numpy as np
import ml_dtypes

import concourse.bass as bass
import concourse.mybir as mybir
from concourse.bass_utils import run_bass_kernel_spmd

# ---------------------------------------------------------------------------
# host-side kernel construction (tiny: 5x256 weights -> the conv matrices)
# ---------------------------------------------------------------------------
MUL = 8
SIZE = 5
NUM_RBF = SIZE
PW_0E = (1.0 / 16.0) ** 0.5
PW_1O = (3.0 / 16.0) ** 0.5
INV_SQRT3 = 1.0 / (3.0 ** 0.5)
INV_SQRT_MUL = 1.0 / (MUL ** 0.5)


def _soft_unit_step(t):
    safe = np.where(t > 0, t, 1.0)
    return np.where(t > 0, np.exp(-1.0 / safe), 0.0)


def _build_K5(tp_weight):
    """(O,I,D,H,W) = (32,32,5,5,5) conv kernel, fp64."""
    tp_weight = np.asarray(tp_weight, np.float64)
    r = np.linspace(-1.0, 1.0, SIZE)
    lattice = np.stack(np.meshgrid(r, r, r, indexing='ij'), axis=-1)
    norm = np.linalg.norm(lattice, axis=-1)
    values = np.linspace(0.0, 1.0, NUM_RBF + 2)[1:-1]
    step = 1.0 / (NUM_RBF + 1)
    diff = (norm[..., None] - values) / step
    emb = 1.14136 * np.e ** 2 * _soft_unit_step(diff + 1.0) * _soft_unit_step(1.0 - diff)
    unit = lattice / np.maximum(norm, 1e-12)[..., None]
    sh1 = (3.0 ** 0.5) * unit[..., [1, 2, 0]]
    w = (emb @ tp_weight) / SIZE ** 1.5
    W1, W2, W3, W4 = [w[..., i * 64:(i + 1) * 64].reshape(SIZE, SIZE, SIZE, MUL, MUL)
                      for i in range(4)]
    B00 = PW_0E * W1
    B01 = (PW_1O * INV_SQRT3) * np.einsum('...uw,...k->...uwk', W2, sh1)
    B01 = B01.reshape(SIZE, SIZE, SIZE, MUL, 3 * MUL)
    B10 = (PW_0E * INV_SQRT3) * np.einsum('...uw,...i->...uiw', W4, sh1)
    B10 = B10.reshape(SIZE, SIZE, SIZE, 3 * MUL, MUL)
    B11 = (PW_1O * INV_SQRT3) * np.einsum('...uw,ik->...uiwk', W3, np.eye(3))
    B11 = B11.reshape(SIZE, SIZE, SIZE, 3 * MUL, 3 * MUL)
    K = np.concatenate([np.concatenate([B00, B01], axis=-1),
                        np.concatenate([B10, B11], axis=-1)], axis=-2)
    return K.transpose(4, 3, 0, 1, 2)


def _build_parts(tp_weight, w_sc0, w_sc1):
    """Returns (W12, Wc): W12 = 12 block-Toeplitz (128,128) matrices for the
    26 outer taps (center zeroed), Wc = (O,I) center tap + self-connection.

    W12 layout: rows p = ci*4 + zi*2 + hi (contraction), cols m =
    co*4 + zo*2 + ho.  t = dxi*4 + bz*2 + bh; dz = zi - zo + 2bz - 1,
    dh analogous, dw = dxi - 1.
    """
    K5 = _build_K5(tp_weight)
    K3 = 0.1 * K5[:, :, 1:4, 1:4, 1:4]
    Wsc = np.zeros((32, 32))
    Wsc[:8, :8] = INV_SQRT_MUL * np.asarray(w_sc0, np.float64)
    for i in range(3):
        Wsc[8 + i::3, 8 + i::3][:8, :8] = INV_SQRT_MUL * np.asarray(w_sc1, np.float64)
    Wc = K3[:, :, 1, 1, 1] + Wsc.T
    K3o = K3.copy()
    K3o[:, :, 1, 1, 1] = 0.0
    W = np.zeros((12, 128, 128))
    for dxi in range(3):
        for bz in range(2):
            for bh in range(2):
                t = dxi * 4 + bz * 2 + bh
                for zi in range(2):
                    for hi in range(2):
                        for zo in range(2):
                            for ho in range(2):
                                dz = zi - zo + 2 * bz - 1
                                dh = hi - ho + 2 * bh - 1
                                if abs(dz) <= 1 and abs(dh) <= 1:
                                    W[t, zi * 2 + hi::4, zo * 2 + ho::4] = \
                                        K3o[:, :, dz + 1, dh + 1, dxi].T
    return W, Wc


def _pack_w8(W12s):
    """(12,128,128) scaled fp64 -> (128, 6, 2, 128) e4m3.
    pair pr = dxi*2 + bh, k-tile dim = bz."""
    a = W12s.reshape(3, 2, 2, 128, 128).transpose(0, 2, 1, 3, 4)
    a = a.reshape(6, 2, 128, 128).transpose(2, 0, 1, 3)
    return np.ascontiguousarray(a).astype(ml_dtypes.float8_e4m3)


def _pack_wc(Wc, scale):
    """(O,I) center+sc -> (128,128) bf16 block-diagonal over the 4 voxel
    positions: wc[ci*4+q, co*4+q] = scale * Wc[co, ci]."""
    scaled = scale * Wc.T
    wcb = np.zeros((128, 128))
    for q in range(4):
        wcb[q::4, q::4] = scaled
    return np.ascontiguousarray(wcb).astype(ml_dtypes.bfloat16)


def _shard_input8(x8, core):
    """(128, 17, 33, 66) e4m3, partition-blocked and zero-padded.
    x8: full input already scaled+cast to e4m3.  core = 2*batch + d_half.
    p = ci*4 + zi*2 + hi; free (zb, hb, w) with z_in_local = 2*zb - 1 + zi,
    h = 2*hb - 1 + hi, w stored at w+1."""
    n, half = core // 2, core % 2
    d0 = 32 * half
    xp = np.zeros((32, 34, 66, 66), ml_dtypes.float8_e4m3)
    lo, hi = d0 - 1, d0 + 33
    glo, ghi = max(lo, 0), min(hi, 64)
    xp[:, glo - lo:ghi - lo, 1:65, 1:65] = x8[n, :, glo:ghi]
    return np.ascontiguousarray(
        xp.reshape(32, 17, 2, 33, 2, 66).transpose(0, 2, 4, 1, 3, 5)
        .reshape(128, 17, 33, 66))


def _shard_aligned(x16, core):
    """(128, 16, 2048) fp16, aligned blocking (no halo): p = ci*4+zi*2+hi,
    free (zb, hb*64+w) with z = 2*zb+zi, h = 2*hb+hi."""
    n, half = core // 2, core % 2
    d0 = 32 * half
    blk = x16[n, :, d0:d0 + 32]
    return np.ascontiguousarray(
        blk.reshape(32, 16, 2, 32, 2, 64).transpose(0, 2, 4, 1, 3, 5)
        .reshape(128, 16, 2048))


def _unshard_output(core_outs, descale):
    out = np.empty((4, 32, 64, 64, 64), np.float32)
    for c, oc in enumerate(core_outs):
        blk = np.asarray(oc).astype(np.float32) * descale
        blk = (blk.reshape(32, 2, 2, 16, 32, 64)
               .transpose(0, 3, 1, 4, 2, 5).reshape(32, 32, 64, 64))
        out[c // 2, :, 32 * (c % 2):32 * (c % 2) + 32] = blk
    return out


# ---------------------------------------------------------------------------
# device kernel
# ---------------------------------------------------------------------------
def _build_nc():
    """Raw-bass kernel (explicit semaphores).

    Engine streams: SP runs all DMAs (HWDGE), PE runs 448 matmuls
    (64 tiles x [1 bf16 center + 6 fp8 DoubleRow]), ACT evicts PSUM->SBUF
    as bf16.  Pipeline: 8 PSUM banks (2 zb groups) in flight; 8 output
    staging slots.  Per zb the 4 center matmuls are grouped before the 24
    DoubleRow matmuls to minimize PE dtype-mode switches."""
    nc = bass.Bass()
    x8 = nc.declare_dram_parameter("x8_sh", [128, 17, 33, 66], mybir.dt.float8e4,
                                   isOutput=False)
    xal = nc.declare_dram_parameter("xal_sh", [128, 16, 2048], mybir.dt.float16,
                                    isOutput=False)
    w8 = nc.declare_dram_parameter("w8", [128, 6, 2, 128], mybir.dt.float8e4,
                                   isOutput=False)
    wc = nc.declare_dram_parameter("wc", [128, 128], mybir.dt.bfloat16,
                                   isOutput=False)
    out = nc.declare_dram_parameter("out_sh", [128, 16, 32, 64], mybir.dt.bfloat16,
                                    isOutput=True)
    # Per-DMA semaphores: a DMA's completion increments its own sem by 16
    # (one per shard across the 16 HWDGE queues).  A shared counting sem
    # is racy: shard counts from a LATER dma can satisfy a threshold while
    # an earlier dma still has shards in flight on a lagging queue.
    wc_sem = nc.alloc_semaphore("wc_sem")
    w8_sem = nc.alloc_semaphore("w8_sem")
    xs_sem = [nc.alloc_semaphore(f"xs_sem{j}") for j in range(17)]
    al_sem = [nc.alloc_semaphore(f"al_sem{j}") for j in range(16)]
    od_sem = [nc.alloc_semaphore(f"od_sem{j}") for j in range(16)]
    warm_sem = nc.alloc_semaphore("warm_sem")
    N_WARM = 5  # PE pstate warmup matmuls: end right as the first data lands
    with (
        nc.sbuf_tensor([128, 6, 2, 128], mybir.dt.float8e4) as wt8,
        nc.sbuf_tensor([128, 128], mybir.dt.bfloat16) as wtc,
        nc.sbuf_tensor([128, 17, 33, 66], mybir.dt.float8e4) as xsb,
        nc.sbuf_tensor([128, 16, 2048], mybir.dt.float16) as xab,
        nc.sbuf_tensor([128, 8, 2048], mybir.dt.bfloat16) as ot,
        nc.psum_tensor([128, 8, 512], mybir.dt.float32) as ps,
        nc.semaphore("pe_sem") as pe_sem,
        nc.semaphore("act_sem") as act_sem,
        nc.Block() as block,
    ):
        @block.vector
        def _(vector):
            # init the PE warmup scratch (ot slot 0); DVE is otherwise idle
            nc.vector.memset(ot[:, 0, :512], 0.0).then_inc(warm_sem, 1)

        @block.sync
        def _(sync):
            # order: first-center deps (wc, xal0), then first-DR deps
            sync.dma_start(out=wtc[:], in_=wc[:]).then_inc(wc_sem, 16)
            sync.dma_start(out=xab[:, 0], in_=xal[:, 0]).then_inc(al_sem[0], 16)
            sync.dma_start(out=wt8[:], in_=w8[:]).then_inc(w8_sem, 16)
            sync.dma_start(out=xsb[:, 0], in_=x8[:, 0]).then_inc(xs_sem[0], 16)
            sync.dma_start(out=xsb[:, 1], in_=x8[:, 1]).then_inc(xs_sem[1], 16)
            for j in range(1, 16):
                sync.dma_start(out=xab[:, j],
                               in_=xal[:, j]).then_inc(al_sem[j], 16)
                sync.dma_start(out=xsb[:, j + 1],
                               in_=x8[:, j + 1]).then_inc(xs_sem[j + 1], 16)
            # per-ht output pieces: the last 128KB piece chases the last
            # eviction instead of a whole 512KB chunk trailing it
            for zb in range(16):
                for ht in range(4):
                    sync.wait_ge(act_sem, 4 * zb + ht + 1)
                    sync.dma_start(
                        out=out[:, zb, ht * 8:(ht + 1) * 8],
                        in_=ot[:, zb % 8, ht * 512:(ht + 1) * 512],
                    ).then_inc(od_sem[zb], 16)
            for zb in range(16):
                sync.wait_ge(od_sem[zb], 64)

        @block.tensor
        def _(tensor):
            # pstate warmup on the ACT-memset scratch (ot slot 0); results
            # land in psum bank 0 and are overwritten by tile 0's start=True.
            tensor.wait_ge(warm_sem, 1)
            for _ in range(N_WARM):
                nc.tensor.matmul(ps[:, 0], lhsT=ot[:, 0, :128],
                                 rhs=ot[:, 0, :512], start=True, stop=True,
                                 skip_group_check=True)
            for zb in range(16):
                if zb == 0:
                    tensor.wait_ge(wc_sem, 16)
                tensor.wait_ge(al_sem[zb], 16)
                for ht in range(4):
                    i = zb * 4 + ht
                    if i >= 8:
                        # PSUM bank reuse: eviction of tile i-8 done
                        tensor.wait_ge(act_sem, i - 7)
                    nc.tensor.matmul(ps[:, i % 8], lhsT=wtc[:],
                                     rhs=xab[:, zb, ht * 512:(ht + 1) * 512],
                                     start=True, stop=False)
                if zb == 0:
                    tensor.wait_ge(w8_sem, 16)
                    tensor.wait_ge(xs_sem[0], 16)
                tensor.wait_ge(xs_sem[zb + 1], 16)
                for ht in range(4):
                    i = zb * 4 + ht
                    hb0 = ht * 8
                    for pr in range(6):
                        dxi, bh = pr // 2, pr % 2
                        dx = dxi - 1
                        rhs = xsb[:, zb:zb + 2, hb0 + bh:hb0 + bh + 8,
                                  1 + dx:65 + dx]
                        mm = nc.tensor.matmul(
                            ps[:, i % 8], lhsT=wt8[:, pr], rhs=rhs,
                            start=False, stop=(pr == 5),
                            perf_mode=mybir.MatmulPerfMode.DoubleRow)
                        if pr == 5:
                            mm.then_inc(pe_sem, 1)

        @block.scalar
        def _(scalar):
            for zb in range(16):
                for ht in range(4):
                    i = zb * 4 + ht
                    scalar.wait_ge(pe_sem, i + 1)
                    if ht == 0 and zb >= 8:
                        # output staging slot reuse: DMA of zb-8 done
                        scalar.wait_ge(od_sem[zb - 8], 64)
                    nc.scalar.copy(
                        out=ot[:, zb % 8, ht * 512:(ht + 1) * 512],
                        in_=ps[:, i % 8]).then_inc(act_sem, 1)

    return nc


_NC_CACHE = None


def _get_nc():
    global _NC_CACHE
    if _NC_CACHE is None:
        _NC_CACHE = _build_nc()
    return _NC_CACHE


def _pow2_scale(absmax, target=192.0):
    return float(2.0 ** np.floor(np.log2(target / max(absmax, 1e-30))))


def _prepare_inputs(x, tp_weight, w_sc0, w_sc1):
    """Returns (in_maps, descale)."""
    x = np.asarray(x, np.float32)
    s_x = _pow2_scale(float(np.abs(x).max()))
    W12, Wc = _build_parts(tp_weight, w_sc0, w_sc1)
    s_w = _pow2_scale(float(np.abs(W12).max()))
    w8 = _pack_w8(W12 * s_w)
    wcq = _pack_wc(Wc, s_w * s_x)
    x8full = (x * s_x).astype(ml_dtypes.float8_e4m3)
    x16full = x.astype(np.float16)
    in_maps = [{"x8_sh": _shard_input8(x8full, c),
                "xal_sh": _shard_aligned(x16full, c),
                "w8": w8, "wc": wcq} for c in range(8)]
    return in_maps, 1.0 / (s_w * s_x)


def run_sharded(x, tp_weight, w_sc0, w_sc1, **run_kwargs):
    """Shard, run on 8 cores, unshard.  Returns (output, BassKernelResults)."""
    in_maps, descale = _prepare_inputs(x, tp_weight, w_sc0, w_sc1)
    nc = _get_nc()
    res = run_bass_kernel_spmd(nc, in_maps, list(range(8)), **run_kwargs)
    outs = [res.results[c]["out_sh"] for c in range(8)]
    return _unshard_output(outs, descale), res


def kernel(x, tp_weight, w_sc0, w_sc1):
    out, _ = run_sharded(x, tp_weight, w_sc0, w_sc1)
    return out


# revision 14
# speedup vs baseline: 1.0368x; 1.0143x over previous
"""Trainium2 Bass kernel for nn_ConvolutionVoxel (e3nn voxel convolution).

The module computes  sc(x) + 0.1 * conv3d(x, K(tp_weight))  with a 5x5x5
e3nn tensor-product kernel on a (4, 32, 64, 64, 64) voxel grid.

Key observations exploited (all verified numerically):
  * The smooth-finite RBF basis has a hard cutoff at norm >= 1 and the
    kernel lattice spans [-1,1]^3, so the effective kernel is the central
    3x3x3 (26 nonzero taps) and the center tap is zero.
  * The self-connection is a 1x1x1 channel mix folded into the center tap.
  * Output = (large) center/self-connection part + (small, 0.1-scaled)
    26-tap part.  The small part tolerates fp8 weights/activations
    (max rel err contribution ~1e-3); the large part is done in bf16/fp16.

Mapping to the PE array (per core): contraction = 32 in-ch x (2z x 2h)
voxel block = 128, out = 32 out-ch x (2z x 2h) block = 128.  The 3 W-taps
are free-dim shifts of the rhs; z/h taps decompose into 2x2 neighbor
in-blocks.  The 12 resulting matmuls pair over bz into 6 fp8 DoubleRow
matmuls (contraction 256, 2x throughput), plus 1 bf16 matmul on an
*aligned* fp16 copy of x for the center tap + self-connection.
7 x 512 PE rows per tile instead of the all-bf16 12 x 512.

Sharding: 8 cores = 4 batches x 2 D-halves; 1 halo plane via host-side
zero padding (full inputs are given, so no device collectives needed).
"""
import numpy as np
import ml_dtypes

import concourse.bass as bass
import concourse.mybir as mybir
from concourse.bass_utils import run_bass_kernel_spmd

# ---------------------------------------------------------------------------
# host-side kernel construction (tiny: 5x256 weights -> the conv matrices)
# ---------------------------------------------------------------------------
MUL = 8
SIZE = 5
NUM_RBF = SIZE
PW_0E = (1.0 / 16.0) ** 0.5
PW_1O = (3.0 / 16.0) ** 0.5
INV_SQRT3 = 1.0 / (3.0 ** 0.5)
INV_SQRT_MUL = 1.0 / (MUL ** 0.5)


def _soft_unit_step(t):
    safe = np.where(t > 0, t, 1.0)
    return np.where(t > 0, np.exp(-1.0 / safe), 0.0)


def _build_K5(tp_weight):
    """(O,I,D,H,W) = (32,32,5,5,5) conv kernel, fp64."""
    tp_weight = np.asarray(tp_weight, np.float64)
    r = np.linspace(-1.0, 1.0, SIZE)
    lattice = np.stack(np.meshgrid(r, r, r, indexing='ij'), axis=-1)
    norm = np.linalg.norm(lattice, axis=-1)
    values = np.linspace(0.0, 1.0, NUM_RBF + 2)[1:-1]
    step = 1.0 / (NUM_RBF + 1)
    diff = (norm[..., None] - values) / step
    emb = 1.14136 * np.e ** 2 * _soft_unit_step(diff + 1.0) * _soft_unit_step(1.0 - diff)
    unit = lattice / np.maximum(norm, 1e-12)[..., None]
    sh1 = (3.0 ** 0.5) * unit[..., [1, 2, 0]]
    w = (emb @ tp_weight) / SIZE ** 1.5
    W1, W2, W3, W4 = [w[..., i * 64:(i + 1) * 64].reshape(SIZE, SIZE, SIZE, MUL, MUL)
                      for i in range(4)]
    B00 = PW_0E * W1
    B01 = (PW_1O * INV_SQRT3) * np.einsum('...uw,...k->...uwk', W2, sh1)
    B01 = B01.reshape(SIZE, SIZE, SIZE, MUL, 3 * MUL)
    B10 = (PW_0E * INV_SQRT3) * np.einsum('...uw,...i->...uiw', W4, sh1)
    B10 = B10.reshape(SIZE, SIZE, SIZE, 3 * MUL, MUL)
    B11 = (PW_1O * INV_SQRT3) * np.einsum('...uw,ik->...uiwk', W3, np.eye(3))
    B11 = B11.reshape(SIZE, SIZE, SIZE, 3 * MUL, 3 * MUL)
    K = np.concatenate([np.concatenate([B00, B01], axis=-1),
                        np.concatenate([B10, B11], axis=-1)], axis=-2)
    return K.transpose(4, 3, 0, 1, 2)


def _build_parts(tp_weight, w_sc0, w_sc1):
    """Returns (W12, Wc): W12 = 12 block-Toeplitz (128,128) matrices for the
    26 outer taps (center zeroed), Wc = (O,I) center tap + self-connection.

    W12 layout: rows p = ci*4 + zi*2 + hi (contraction), cols m =
    co*4 + zo*2 + ho.  t = dxi*4 + bz*2 + bh; dz = zi - zo + 2bz - 1,
    dh analogous, dw = dxi - 1.
    """
    K5 = _build_K5(tp_weight)
    K3 = 0.1 * K5[:, :, 1:4, 1:4, 1:4]
    Wsc = np.zeros((32, 32))
    Wsc[:8, :8] = INV_SQRT_MUL * np.asarray(w_sc0, np.float64)
    for i in range(3):
        Wsc[8 + i::3, 8 + i::3][:8, :8] = INV_SQRT_MUL * np.asarray(w_sc1, np.float64)
    Wc = K3[:, :, 1, 1, 1] + Wsc.T
    K3o = K3.copy()
    K3o[:, :, 1, 1, 1] = 0.0
    W = np.zeros((12, 128, 128))
    for dxi in range(3):
        for bz in range(2):
            for bh in range(2):
                t = dxi * 4 + bz * 2 + bh
                for zi in range(2):
                    for hi in range(2):
                        for zo in range(2):
                            for ho in range(2):
                                dz = zi - zo + 2 * bz - 1
                                dh = hi - ho + 2 * bh - 1
                                if abs(dz) <= 1 and abs(dh) <= 1:
                                    W[t, zi * 2 + hi::4, zo * 2 + ho::4] = \
                                        K3o[:, :, dz + 1, dh + 1, dxi].T
    return W, Wc


def _pack_w8(W12s):
    """(12,128,128) scaled fp64 -> (128, 6, 2, 128) e4m3.
    pair pr = dxi*2 + bh, k-tile dim = bz."""
    a = W12s.reshape(3, 2, 2, 128, 128).transpose(0, 2, 1, 3, 4)
    a = a.reshape(6, 2, 128, 128).transpose(2, 0, 1, 3)
    return np.ascontiguousarray(a).astype(ml_dtypes.float8_e4m3)


def _pack_wc(Wc, scale):
    """(O,I) center+sc -> (128,128) bf16 block-diagonal over the 4 voxel
    positions: wc[ci*4+q, co*4+q] = scale * Wc[co, ci]."""
    scaled = scale * Wc.T
    wcb = np.zeros((128, 128))
    for q in range(4):
        wcb[q::4, q::4] = scaled
    return np.ascontiguousarray(wcb).astype(ml_dtypes.bfloat16)


def _shard_input8(x8, core):
    """(128, 17, 33, 66) e4m3, partition-blocked and zero-padded.
    x8: full input already scaled+cast to e4m3.  core = 2*batch + d_half.
    p = ci*4 + zi*2 + hi; free (zb, hb, w) with z_in_local = 2*zb - 1 + zi,
    h = 2*hb - 1 + hi, w stored at w+1."""
    n, half = core // 2, core % 2
    d0 = 32 * half
    xp = np.zeros((32, 34, 66, 66), ml_dtypes.float8_e4m3)
    lo, hi = d0 - 1, d0 + 33
    glo, ghi = max(lo, 0), min(hi, 64)
    xp[:, glo - lo:ghi - lo, 1:65, 1:65] = x8[n, :, glo:ghi]
    return np.ascontiguousarray(
        xp.reshape(32, 17, 2, 33, 2, 66).transpose(0, 2, 4, 1, 3, 5)
        .reshape(128, 17, 33, 66))


def _shard_aligned(x16, core):
    """(128, 16, 2048) fp16, aligned blocking (no halo): p = ci*4+zi*2+hi,
    free (zb, hb*64+w) with z = 2*zb+zi, h = 2*hb+hi."""
    n, half = core // 2, core % 2
    d0 = 32 * half
    blk = x16[n, :, d0:d0 + 32]
    return np.ascontiguousarray(
        blk.reshape(32, 16, 2, 32, 2, 64).transpose(0, 2, 4, 1, 3, 5)
        .reshape(128, 16, 2048))


def _unshard_output(core_outs, descale):
    out = np.empty((4, 32, 64, 64, 64), np.float32)
    for c, oc in enumerate(core_outs):
        blk = np.asarray(oc).astype(np.float32) * descale
        blk = (blk.reshape(32, 2, 2, 16, 32, 64)
               .transpose(0, 3, 1, 4, 2, 5).reshape(32, 32, 64, 64))
        out[c // 2, :, 32 * (c % 2):32 * (c % 2) + 32] = blk
    return out


# ---------------------------------------------------------------------------
# device kernel
# ---------------------------------------------------------------------------
def _build_nc():
    """Raw-bass kernel (explicit semaphores).

    Engine streams: SP runs all DMAs (HWDGE), PE runs 448 matmuls
    (64 tiles x [1 bf16 center + 6 fp8 DoubleRow]), ACT evicts PSUM->SBUF
    as bf16.  Pipeline: 8 PSUM banks (2 zb groups) in flight; 8 output
    staging slots.  Per zb the 4 center matmuls are grouped before the 24
    DoubleRow matmuls to minimize PE dtype-mode switches."""
    nc = bass.Bass()
    x8 = nc.declare_dram_parameter("x8_sh", [128, 17, 33, 66], mybir.dt.float8e4,
                                   isOutput=False)
    xal = nc.declare_dram_parameter("xal_sh", [128, 16, 2048], mybir.dt.float16,
                                    isOutput=False)
    w8 = nc.declare_dram_parameter("w8", [128, 6, 2, 128], mybir.dt.float8e4,
                                   isOutput=False)
    wc = nc.declare_dram_parameter("wc", [128, 128], mybir.dt.bfloat16,
                                   isOutput=False)
    out = nc.declare_dram_parameter("out_sh", [128, 16, 32, 64], mybir.dt.bfloat16,
                                    isOutput=True)
    # Per-DMA semaphores: a DMA's completion increments its own sem by 16
    # (one per shard across the 16 HWDGE queues).  A shared counting sem
    # is racy: shard counts from a LATER dma can satisfy a threshold while
    # an earlier dma still has shards in flight on a lagging queue.
    wc_sem = nc.alloc_semaphore("wc_sem")
    w8_sem = nc.alloc_semaphore("w8_sem")
    xs_sem = [nc.alloc_semaphore(f"xs_sem{j}") for j in range(17)]
    al_sem = [nc.alloc_semaphore(f"al_sem{j}") for j in range(16)]
    od_sem = [nc.alloc_semaphore(f"od_sem{j}") for j in range(16)]
    warm_sem = nc.alloc_semaphore("warm_sem")
    # PE pstate warmup: keep the PE continuously busy from engine boot
    # (~8.8us) until the first real data lands, so the 2.4GHz pstate is
    # reached before real work and never decays in a DMA-wait gap.
    # Measured: wc+xal[0] land ~12.6us (N_WARM spans boot->12.6),
    # w8+xs[0..1] land ~15.7us (N_FILL spans the centers->DR gap).
    N_WARM = 9
    N_FILL = 10
    with (
        nc.sbuf_tensor([128, 6, 2, 128], mybir.dt.float8e4) as wt8,
        nc.sbuf_tensor([128, 128], mybir.dt.bfloat16) as wtc,
        nc.sbuf_tensor([128, 17, 33, 66], mybir.dt.float8e4) as xsb,
        nc.sbuf_tensor([128, 16, 2048], mybir.dt.float16) as xab,
        nc.sbuf_tensor([128, 8, 2048], mybir.dt.bfloat16) as ot,
        nc.psum_tensor([128, 8, 512], mybir.dt.float32) as ps,
        nc.semaphore("pe_sem") as pe_sem,
        nc.semaphore("act_sem") as act_sem,
        nc.Block() as block,
    ):
        @block.vector
        def _(vector):
            # init the PE warmup scratch (ot slot 0); DVE is otherwise idle
            nc.vector.memset(ot[:, 0, :512], 0.0).then_inc(warm_sem, 1)

        @block.sync
        def _(sync):
            # order: first-center deps (wc, xal0), then first-DR deps
            sync.dma_start(out=wtc[:], in_=wc[:]).then_inc(wc_sem, 16)
            sync.dma_start(out=xab[:, 0], in_=xal[:, 0]).then_inc(al_sem[0], 16)
            sync.dma_start(out=wt8[:], in_=w8[:]).then_inc(w8_sem, 16)
            sync.dma_start(out=xsb[:, 0], in_=x8[:, 0]).then_inc(xs_sem[0], 16)
            sync.dma_start(out=xsb[:, 1], in_=x8[:, 1]).then_inc(xs_sem[1], 16)
            for j in range(1, 16):
                sync.dma_start(out=xab[:, j],
                               in_=xal[:, j]).then_inc(al_sem[j], 16)
                sync.dma_start(out=xsb[:, j + 1],
                               in_=x8[:, j + 1]).then_inc(xs_sem[j + 1], 16)
            # per-ht output pieces: the last 128KB piece chases the last
            # eviction instead of a whole 512KB chunk trailing it
            for zb in range(16):
                for ht in range(4):
                    sync.wait_ge(act_sem, 4 * zb + ht + 1)
                    sync.dma_start(
                        out=out[:, zb, ht * 8:(ht + 1) * 8],
                        in_=ot[:, zb % 8, ht * 512:(ht + 1) * 512],
                    ).then_inc(od_sem[zb], 16)
            for zb in range(16):
                sync.wait_ge(od_sem[zb], 64)

        @block.tensor
        def _(tensor):
            # pstate warmup on the ACT-memset scratch (ot slot 0); results
            # land in psum bank 0 and are overwritten by tile 0's start=True.
            tensor.wait_ge(warm_sem, 1)
            for _ in range(N_WARM):
                nc.tensor.matmul(ps[:, 0], lhsT=ot[:, 0, :128],
                                 rhs=ot[:, 0, :512], start=True, stop=True,
                                 skip_group_check=True)
            for zb in range(16):
                if zb == 0:
                    tensor.wait_ge(wc_sem, 16)
                tensor.wait_ge(al_sem[zb], 16)
                for ht in range(4):
                    i = zb * 4 + ht
                    if i >= 8:
                        # PSUM bank reuse: eviction of tile i-8 done
                        tensor.wait_ge(act_sem, i - 7)
                    nc.tensor.matmul(ps[:, i % 8], lhsT=wtc[:],
                                     rhs=xab[:, zb, ht * 512:(ht + 1) * 512],
                                     start=True, stop=False)
                if zb == 0:
                    # bridge the centers->DR-data gap; bank 7 is safe until
                    # tile 7's start=True center resets it
                    for _ in range(N_FILL):
                        nc.tensor.matmul(ps[:, 7], lhsT=ot[:, 0, :128],
                                         rhs=ot[:, 0, :512], start=True,
                                         stop=True, skip_group_check=True)
                    tensor.wait_ge(w8_sem, 16)
                    tensor.wait_ge(xs_sem[0], 16)
                tensor.wait_ge(xs_sem[zb + 1], 16)
                for ht in range(4):
                    i = zb * 4 + ht
                    hb0 = ht * 8
                    for pr in range(6):
                        dxi, bh = pr // 2, pr % 2
                        dx = dxi - 1
                        rhs = xsb[:, zb:zb + 2, hb0 + bh:hb0 + bh + 8,
                                  1 + dx:65 + dx]
                        mm = nc.tensor.matmul(
                            ps[:, i % 8], lhsT=wt8[:, pr], rhs=rhs,
                            start=False, stop=(pr == 5),
                            perf_mode=mybir.MatmulPerfMode.DoubleRow)
                        if pr == 5:
                            mm.then_inc(pe_sem, 1)

        @block.scalar
        def _(scalar):
            for zb in range(16):
                for ht in range(4):
                    i = zb * 4 + ht
                    scalar.wait_ge(pe_sem, i + 1)
                    if ht == 0 and zb >= 8:
                        # output staging slot reuse: DMA of zb-8 done
                        scalar.wait_ge(od_sem[zb - 8], 64)
                    nc.scalar.copy(
                        out=ot[:, zb % 8, ht * 512:(ht + 1) * 512],
                        in_=ps[:, i % 8]).then_inc(act_sem, 1)

    return nc


_NC_CACHE = None


def _get_nc():
    global _NC_CACHE
    if _NC_CACHE is None:
        _NC_CACHE = _build_nc()
    return _NC_CACHE


def _pow2_scale(absmax, target=192.0):
    return float(2.0 ** np.floor(np.log2(target / max(absmax, 1e-30))))


def _prepare_inputs(x, tp_weight, w_sc0, w_sc1):
    """Returns (in_maps, descale)."""
    x = np.asarray(x, np.float32)
    s_x = _pow2_scale(float(np.abs(x).max()))
    W12, Wc = _build_parts(tp_weight, w_sc0, w_sc1)
    s_w = _pow2_scale(float(np.abs(W12).max()))
    w8 = _pack_w8(W12 * s_w)
    wcq = _pack_wc(Wc, s_w * s_x)
    x8full = (x * s_x).astype(ml_dtypes.float8_e4m3)
    x16full = x.astype(np.float16)
    in_maps = [{"x8_sh": _shard_input8(x8full, c),
                "xal_sh": _shard_aligned(x16full, c),
                "w8": w8, "wc": wcq} for c in range(8)]
    return in_maps, 1.0 / (s_w * s_x)


def run_sharded(x, tp_weight, w_sc0, w_sc1, **run_kwargs):
    """Shard, run on 8 cores, unshard.  Returns (output, BassKernelResults)."""
    in_maps, descale = _prepare_inputs(x, tp_weight, w_sc0, w_sc1)
    nc = _get_nc()
    res = run_bass_kernel_spmd(nc, in_maps, list(range(8)), **run_kwargs)
    outs = [res.results[c]["out_sh"] for c in range(8)]
    return _unshard_output(outs, descale), res


def kernel(x, tp_weight, w_sc0, w_sc1):
    out, _ = run_sharded(x, tp_weight, w_sc0, w_sc1)
    return out


# revision 17
# speedup vs baseline: 1.0547x; 1.0173x over previous
"""Trainium2 Bass kernel for nn_ConvolutionVoxel (e3nn voxel convolution).

The module computes  sc(x) + 0.1 * conv3d(x, K(tp_weight))  with a 5x5x5
e3nn tensor-product kernel on a (4, 32, 64, 64, 64) voxel grid.

Key observations exploited (all verified numerically):
  * The smooth-finite RBF basis has a hard cutoff at norm >= 1 and the
    kernel lattice spans [-1,1]^3, so the effective kernel is the central
    3x3x3 (26 nonzero taps) and the center tap is zero.
  * The self-connection is a 1x1x1 channel mix folded into the center tap.
  * Output = (large) center/self-connection part + (small, 0.1-scaled)
    26-tap part.  The small part tolerates fp8 weights/activations
    (max rel err contribution ~1e-3); the large part is done in bf16/fp16.

Mapping to the PE array (per core): contraction = 32 in-ch x (2z x 2h)
voxel block = 128, out = 32 out-ch x (2z x 2h) block = 128.  The 3 W-taps
are free-dim shifts of the rhs; z/h taps decompose into 2x2 neighbor
in-blocks.  The 12 resulting matmuls pair over bz into 6 fp8 DoubleRow
matmuls (contraction 256, 2x throughput), plus 1 bf16 matmul on an
*aligned* fp16 copy of x for the center tap + self-connection.
7 x 512 PE rows per tile instead of the all-bf16 12 x 512.

Sharding: 8 cores = 4 batches x 2 D-halves; 1 halo plane via host-side
zero padding (full inputs are given, so no device collectives needed).
"""
import numpy as np
import ml_dtypes

import concourse.bass as bass
import concourse.mybir as mybir
from concourse.bass_utils import run_bass_kernel_spmd

# ---------------------------------------------------------------------------
# host-side kernel construction (tiny: 5x256 weights -> the conv matrices)
# ---------------------------------------------------------------------------
MUL = 8
SIZE = 5
NUM_RBF = SIZE
PW_0E = (1.0 / 16.0) ** 0.5
PW_1O = (3.0 / 16.0) ** 0.5
INV_SQRT3 = 1.0 / (3.0 ** 0.5)
INV_SQRT_MUL = 1.0 / (MUL ** 0.5)


def _soft_unit_step(t):
    safe = np.where(t > 0, t, 1.0)
    return np.where(t > 0, np.exp(-1.0 / safe), 0.0)


def _build_K5(tp_weight):
    """(O,I,D,H,W) = (32,32,5,5,5) conv kernel, fp64."""
    tp_weight = np.asarray(tp_weight, np.float64)
    r = np.linspace(-1.0, 1.0, SIZE)
    lattice = np.stack(np.meshgrid(r, r, r, indexing='ij'), axis=-1)
    norm = np.linalg.norm(lattice, axis=-1)
    values = np.linspace(0.0, 1.0, NUM_RBF + 2)[1:-1]
    step = 1.0 / (NUM_RBF + 1)
    diff = (norm[..., None] - values) / step
    emb = 1.14136 * np.e ** 2 * _soft_unit_step(diff + 1.0) * _soft_unit_step(1.0 - diff)
    unit = lattice / np.maximum(norm, 1e-12)[..., None]
    sh1 = (3.0 ** 0.5) * unit[..., [1, 2, 0]]
    w = (emb @ tp_weight) / SIZE ** 1.5
    W1, W2, W3, W4 = [w[..., i * 64:(i + 1) * 64].reshape(SIZE, SIZE, SIZE, MUL, MUL)
                      for i in range(4)]
    B00 = PW_0E * W1
    B01 = (PW_1O * INV_SQRT3) * np.einsum('...uw,...k->...uwk', W2, sh1)
    B01 = B01.reshape(SIZE, SIZE, SIZE, MUL, 3 * MUL)
    B10 = (PW_0E * INV_SQRT3) * np.einsum('...uw,...i->...uiw', W4, sh1)
    B10 = B10.reshape(SIZE, SIZE, SIZE, 3 * MUL, MUL)
    B11 = (PW_1O * INV_SQRT3) * np.einsum('...uw,ik->...uiwk', W3, np.eye(3))
    B11 = B11.reshape(SIZE, SIZE, SIZE, 3 * MUL, 3 * MUL)
    K = np.concatenate([np.concatenate([B00, B01], axis=-1),
                        np.concatenate([B10, B11], axis=-1)], axis=-2)
    return K.transpose(4, 3, 0, 1, 2)


def _build_parts(tp_weight, w_sc0, w_sc1):
    """Returns (W12, Wc): W12 = 12 block-Toeplitz (128,128) matrices for the
    26 outer taps (center zeroed), Wc = (O,I) center tap + self-connection.

    W12 layout: rows p = ci*4 + zi*2 + hi (contraction), cols m =
    co*4 + zo*2 + ho.  t = dxi*4 + bz*2 + bh; dz = zi - zo + 2bz - 1,
    dh analogous, dw = dxi - 1.
    """
    K5 = _build_K5(tp_weight)
    K3 = 0.1 * K5[:, :, 1:4, 1:4, 1:4]
    Wsc = np.zeros((32, 32))
    Wsc[:8, :8] = INV_SQRT_MUL * np.asarray(w_sc0, np.float64)
    for i in range(3):
        Wsc[8 + i::3, 8 + i::3][:8, :8] = INV_SQRT_MUL * np.asarray(w_sc1, np.float64)
    Wc = K3[:, :, 1, 1, 1] + Wsc.T
    K3o = K3.copy()
    K3o[:, :, 1, 1, 1] = 0.0
    W = np.zeros((12, 128, 128))
    for dxi in range(3):
        for bz in range(2):
            for bh in range(2):
                t = dxi * 4 + bz * 2 + bh
                for zi in range(2):
                    for hi in range(2):
                        for zo in range(2):
                            for ho in range(2):
                                dz = zi - zo + 2 * bz - 1
                                dh = hi - ho + 2 * bh - 1
                                if abs(dz) <= 1 and abs(dh) <= 1:
                                    W[t, zi * 2 + hi::4, zo * 2 + ho::4] = \
                                        K3o[:, :, dz + 1, dh + 1, dxi].T
    return W, Wc


def _pack_w8(W12s):
    """(12,128,128) scaled fp64 -> (128, 6, 2, 128) e4m3.
    pair pr = dxi*2 + bh, k-tile dim = bz."""
    a = W12s.reshape(3, 2, 2, 128, 128).transpose(0, 2, 1, 3, 4)
    a = a.reshape(6, 2, 128, 128).transpose(2, 0, 1, 3)
    return np.ascontiguousarray(a).astype(ml_dtypes.float8_e4m3)


def _pack_wc(Wc, scale):
    """(O,I) center+sc -> (128,128) bf16 block-diagonal over the 4 voxel
    positions: wc[ci*4+q, co*4+q] = scale * Wc[co, ci]."""
    scaled = scale * Wc.T
    wcb = np.zeros((128, 128))
    for q in range(4):
        wcb[q::4, q::4] = scaled
    return np.ascontiguousarray(wcb).astype(ml_dtypes.bfloat16)


def _shard_input8(x8, core):
    """(128, 17, 33, 66) e4m3, partition-blocked and zero-padded.
    x8: full input already scaled+cast to e4m3.  core = 2*batch + d_half.
    p = ci*4 + zi*2 + hi; free (zb, hb, w) with z_in_local = 2*zb - 1 + zi,
    h = 2*hb - 1 + hi, w stored at w+1."""
    n, half = core // 2, core % 2
    d0 = 32 * half
    xp = np.zeros((32, 34, 66, 66), ml_dtypes.float8_e4m3)
    lo, hi = d0 - 1, d0 + 33
    glo, ghi = max(lo, 0), min(hi, 64)
    xp[:, glo - lo:ghi - lo, 1:65, 1:65] = x8[n, :, glo:ghi]
    return np.ascontiguousarray(
        xp.reshape(32, 17, 2, 33, 2, 66).transpose(0, 2, 4, 1, 3, 5)
        .reshape(128, 17, 33, 66))


def _shard_aligned(x16, core):
    """(128, 16, 2048) fp16, aligned blocking (no halo): p = ci*4+zi*2+hi,
    free (zb, hb*64+w) with z = 2*zb+zi, h = 2*hb+hi."""
    n, half = core // 2, core % 2
    d0 = 32 * half
    blk = x16[n, :, d0:d0 + 32]
    return np.ascontiguousarray(
        blk.reshape(32, 16, 2, 32, 2, 64).transpose(0, 2, 4, 1, 3, 5)
        .reshape(128, 16, 2048))


def _unshard_output(core_outs, descale):
    out = np.empty((4, 32, 64, 64, 64), np.float32)
    for c, oc in enumerate(core_outs):
        blk = np.asarray(oc).astype(np.float32) * descale
        blk = (blk.reshape(32, 2, 2, 16, 32, 64)
               .transpose(0, 3, 1, 4, 2, 5).reshape(32, 32, 64, 64))
        out[c // 2, :, 32 * (c % 2):32 * (c % 2) + 32] = blk
    return out


# ---------------------------------------------------------------------------
# device kernel
# ---------------------------------------------------------------------------
def _build_nc():
    """Raw-bass kernel (explicit semaphores).

    Engine streams: SP runs all DMAs (HWDGE), PE runs 448 matmuls
    (64 tiles x [1 bf16 center + 6 fp8 DoubleRow]), ACT evicts PSUM->SBUF
    as bf16.  Pipeline: 8 PSUM banks (2 zb groups) in flight; 8 output
    staging slots.  Per zb the 4 center matmuls are grouped before the 24
    DoubleRow matmuls to minimize PE dtype-mode switches."""
    nc = bass.Bass()
    x8 = nc.declare_dram_parameter("x8_sh", [128, 17, 33, 66], mybir.dt.float8e4,
                                   isOutput=False)
    xal = nc.declare_dram_parameter("xal_sh", [128, 16, 2048], mybir.dt.float16,
                                    isOutput=False)
    w8 = nc.declare_dram_parameter("w8", [128, 6, 2, 128], mybir.dt.float8e4,
                                   isOutput=False)
    wc = nc.declare_dram_parameter("wc", [128, 128], mybir.dt.bfloat16,
                                   isOutput=False)
    out = nc.declare_dram_parameter("out_sh", [128, 16, 32, 64], mybir.dt.bfloat16,
                                    isOutput=True)
    # Per-DMA semaphores: a DMA's completion increments its own sem by 16
    # (one per shard across the 16 HWDGE queues).  A shared counting sem
    # is racy: shard counts from a LATER dma can satisfy a threshold while
    # an earlier dma still has shards in flight on a lagging queue.
    wc_sem = nc.alloc_semaphore("wc_sem")
    w8_sem = nc.alloc_semaphore("w8_sem")
    xs_sem = [nc.alloc_semaphore(f"xs_sem{j}") for j in range(17)]
    al_sem = [nc.alloc_semaphore(f"al_sem{j}") for j in range(16)]
    od_sem = [nc.alloc_semaphore(f"od_sem{j}") for j in range(16)]
    warm_sem = nc.alloc_semaphore("warm_sem")
    # PE pstate warmup: keep the PE continuously busy from engine boot
    # (~8.5us) until the first real data lands, so the 2.4GHz pstate is
    # reached before real work and never decays in a DMA-wait gap.
    # Measured: w8+xs[0..1] (the first-DR deps, issued first) land ~13.3us.
    N_WARM = 11
    with (
        nc.sbuf_tensor([128, 6, 2, 128], mybir.dt.float8e4) as wt8,
        nc.sbuf_tensor([128, 128], mybir.dt.bfloat16) as wtc,
        nc.sbuf_tensor([128, 17, 33, 66], mybir.dt.float8e4) as xsb,
        nc.sbuf_tensor([128, 16, 2048], mybir.dt.float16) as xab,
        nc.sbuf_tensor([128, 8, 2048], mybir.dt.bfloat16) as ot,
        nc.psum_tensor([128, 8, 512], mybir.dt.float32) as ps,
        nc.semaphore("pe_sem") as pe_sem,
        nc.semaphore("act_sem") as act_sem,
        nc.Block() as block,
    ):
        @block.vector
        def _(vector):
            # init the PE warmup scratch (ot slot 0); DVE is otherwise idle
            nc.vector.memset(ot[:, 0, :512], 0.0).then_inc(warm_sem, 1)

        @block.sync
        def _(sync):
            # order: first-DR deps (w8, xs0, xs1), then first-center deps
            sync.dma_start(out=wt8[:], in_=w8[:]).then_inc(w8_sem, 16)
            sync.dma_start(out=xsb[:, 0], in_=x8[:, 0]).then_inc(xs_sem[0], 16)
            sync.dma_start(out=xsb[:, 1], in_=x8[:, 1]).then_inc(xs_sem[1], 16)
            sync.dma_start(out=wtc[:], in_=wc[:]).then_inc(wc_sem, 16)
            sync.dma_start(out=xab[:, 0], in_=xal[:, 0]).then_inc(al_sem[0], 16)
            for j in range(1, 16):
                sync.dma_start(out=xsb[:, j + 1],
                               in_=x8[:, j + 1]).then_inc(xs_sem[j + 1], 16)
                sync.dma_start(out=xab[:, j],
                               in_=xal[:, j]).then_inc(al_sem[j], 16)
            # per-ht output pieces: the last 128KB piece chases the last
            # eviction instead of a whole 512KB chunk trailing it
            for zb in range(16):
                for ht in range(4):
                    sync.wait_ge(act_sem, 4 * zb + ht + 1)
                    sync.dma_start(
                        out=out[:, zb, ht * 8:(ht + 1) * 8],
                        in_=ot[:, zb % 8, ht * 512:(ht + 1) * 512],
                    ).then_inc(od_sem[zb], 16)
            for zb in range(16):
                sync.wait_ge(od_sem[zb], 64)

        @block.tensor
        def _(tensor):
            # pstate warmup on the DVE-memset scratch (ot slot 0); results
            # land in psum bank 0 and are overwritten by tile 0's start=True.
            tensor.wait_ge(warm_sem, 1)
            for _ in range(N_WARM):
                nc.tensor.matmul(ps[:, 0], lhsT=ot[:, 0, :128],
                                 rhs=ot[:, 0, :512], start=True, stop=True,
                                 skip_group_check=True)

            def dr_phase(tensor, zb, start):
                for ht in range(4):
                    i = zb * 4 + ht
                    hb0 = ht * 8
                    if start and i >= 8:
                        # PSUM bank reuse: eviction of tile i-8 done
                        tensor.wait_ge(act_sem, i - 7)
                    for pr in range(6):
                        dxi, bh = pr // 2, pr % 2
                        dx = dxi - 1
                        rhs = xsb[:, zb:zb + 2, hb0 + bh:hb0 + bh + 8,
                                  1 + dx:65 + dx]
                        mm = nc.tensor.matmul(
                            ps[:, i % 8], lhsT=wt8[:, pr], rhs=rhs,
                            start=(start and pr == 0),
                            stop=(not start and pr == 5),
                            perf_mode=mybir.MatmulPerfMode.DoubleRow)
                        if not start and pr == 5:
                            mm.then_inc(pe_sem, 1)

            def center_phase(tensor, zb, start):
                for ht in range(4):
                    i = zb * 4 + ht
                    if start and i >= 8:
                        tensor.wait_ge(act_sem, i - 7)
                    mm = nc.tensor.matmul(ps[:, i % 8], lhsT=wtc[:],
                                          rhs=xab[:, zb, ht * 512:(ht + 1) * 512],
                                          start=start, stop=not start)
                    if not start:
                        mm.then_inc(pe_sem, 1)

            for zb in range(16):
                if zb == 0:
                    tensor.wait_ge(w8_sem, 16)
                    tensor.wait_ge(xs_sem[0], 16)
                tensor.wait_ge(xs_sem[zb + 1], 16)
                if zb < 15:
                    # DRs open the PSUM group; the center (whose xal data
                    # arrives later than x8) closes it
                    dr_phase(tensor, zb, start=True)
                    if zb == 0:
                        tensor.wait_ge(wc_sem, 16)
                    tensor.wait_ge(al_sem[zb], 16)
                    center_phase(tensor, zb, start=False)
                else:
                    # last zb: centers first so the final evictions overlap
                    # the DR phase instead of trailing the kernel
                    tensor.wait_ge(al_sem[zb], 16)
                    center_phase(tensor, zb, start=True)
                    dr_phase(tensor, zb, start=False)

        @block.scalar
        def _(scalar):
            for zb in range(16):
                for ht in range(4):
                    i = zb * 4 + ht
                    scalar.wait_ge(pe_sem, i + 1)
                    if ht == 0 and zb >= 8:
                        # output staging slot reuse: DMA of zb-8 done
                        scalar.wait_ge(od_sem[zb - 8], 64)
                    nc.scalar.copy(
                        out=ot[:, zb % 8, ht * 512:(ht + 1) * 512],
                        in_=ps[:, i % 8]).then_inc(act_sem, 1)

    return nc


_NC_CACHE = None


def _get_nc():
    global _NC_CACHE
    if _NC_CACHE is None:
        _NC_CACHE = _build_nc()
    return _NC_CACHE


def _pow2_scale(absmax, target=192.0):
    return float(2.0 ** np.floor(np.log2(target / max(absmax, 1e-30))))


def _prepare_inputs(x, tp_weight, w_sc0, w_sc1):
    """Returns (in_maps, descale)."""
    x = np.asarray(x, np.float32)
    s_x = _pow2_scale(float(np.abs(x).max()))
    W12, Wc = _build_parts(tp_weight, w_sc0, w_sc1)
    s_w = _pow2_scale(float(np.abs(W12).max()))
    w8 = _pack_w8(W12 * s_w)
    wcq = _pack_wc(Wc, s_w * s_x)
    x8full = (x * s_x).astype(ml_dtypes.float8_e4m3)
    x16full = x.astype(np.float16)
    in_maps = [{"x8_sh": _shard_input8(x8full, c),
                "xal_sh": _shard_aligned(x16full, c),
                "w8": w8, "wc": wcq} for c in range(8)]
    return in_maps, 1.0 / (s_w * s_x)


def run_sharded(x, tp_weight, w_sc0, w_sc1, **run_kwargs):
    """Shard, run on 8 cores, unshard.  Returns (output, BassKernelResults)."""
    in_maps, descale = _prepare_inputs(x, tp_weight, w_sc0, w_sc1)
    nc = _get_nc()
    res = run_bass_kernel_spmd(nc, in_maps, list(range(8)), **run_kwargs)
    outs = [res.results[c]["out_sh"] for c in range(8)]
    return _unshard_output(outs, descale), res


def kernel(x, tp_weight, w_sc0, w_sc1):
    out, _ = run_sharded(x, tp_weight, w_sc0, w_sc1)
    return out
